# revision 15
# baseline (speedup 1.0000x reference)
"""CRF forward (log partition) on 8 NeuronCores — chunked-parallel recurrence.

Math: the probability-space recurrence P_{t+1} = G_t o (E @ P_t) contracts
direction exponentially fast (products of positive matrices), so the 512
serial steps are split into C=12 time chunks run as independent streams,
each warm-started ~9 steps early from an all-ones state.  Host-side
stitching recovers log Z from per-chunk boundary row-sum ratios (the warmup
constant cancels); measured direction error after 8 steps is ~1e-5.

Range control without on-device renorm: active emission rows are prescaled
host-side by softmax times e^{-gamma}; the exact correction sum_t (LSE +
gamma) is added back on the host.  Absorbed steps (t >= len) park the
sequence's STOP projection in a dedicated 46th row per group whose
self-transition is exactly 1.0, so parked values are bit-stable in bf16.

Execution: 12 chunks form 2 lockstep cohorts of 6.  A cohort tick is ONE
PE matmul (lhsT = blockdiag(Ebar^T, Ebar^T) bf16, rhs = [92, 6*64] packed
states) and ONE DVE multiply (G-slice o PSUM -> next states), so the
PSUM-access cost and matmul fixed latency amortize over 6 chunks, and the
two cohorts keep PE and DVE pipelined against each other.
"""

import numpy as np
import ml_dtypes

import concourse.bacc as bacc
import concourse.bass as bass
import concourse.mybir as mybir
import concourse.tile as tile
from concourse.bass_utils import run_bass_kernel_spmd

L = 45
START = 43
STOP = 44
LBAR = 46                  # labels + park row
PARK = 45
B = 1024
S = 512
NCORES = 8
BPC = B // NCORES          # 128 sequences per core
NG = 2                     # groups per core
WCOL = BPC // NG           # 64 columns per group
PR = NG * LBAR             # 92 partition rows for packed state
TS = S + 1                 # apps 0..512 (app 0 folded host-side, 512 appended absorb)

C = 12                     # time chunks
NCOH = 2                   # lockstep cohorts
CPC = C // NCOH            # chunks per cohort
TICKS = 51                 # apps per chunk incl warmup
WARM = 9                   # warmup apps (chunks 1..C-1)
# windows: chunk 0 runs apps 1..51 exactly; chunks 1..10 cover 42 apps each,
# chunk 11 covers 41 apps + 1 pad absorb app (exact no-op on parked state).
W0 = TICKS
WC = 42
BOUNDS = [1, 1 + W0] + [1 + W0 + WC * c for c in range(1, C - 1)] + [TS]
assert BOUNDS[-2] + WC >= TS and len(BOUNDS) == C + 1

NSLOT = 2 * C - 1          # 11 start snaps + 12 end snaps
CW = CPC * WCOL            # cohort tile width (384)

F32 = mybir.dt.float32
BF16 = mybir.dt.bfloat16

# DMA pieces per cohort G tensor, in ticks
PIECES = (3, 6, 10, 14, 18)


def _build_nc():
    nc = bacc.Bacc("TRN2", target_bir_lowering=False, debug=False, num_devices=NCORES)
    e2t_dram = nc.dram_tensor("e2t", [PR, PR], BF16, kind="ExternalInput")
    s0_dram = nc.dram_tensor("s0", [PR, WCOL], BF16, kind="ExternalInput")
    g_dram = [
        nc.dram_tensor(f"g{k}", [PR, TICKS * CW], BF16, kind="ExternalInput")
        for k in range(NCOH)
    ]
    snaps_dram = nc.dram_tensor("snaps", [PR, NSLOT * WCOL], BF16,
                                kind="ExternalOutput")

    with tile.TileContext(nc) as tc:
        with (
            tc.tile_pool(name="const", bufs=1) as const_pool,
            tc.tile_pool(name="gtiles", bufs=1) as g_pool,
            tc.tile_pool(name="strip", bufs=1) as strip_pool,
            tc.tile_pool(name="state", bufs=3) as state_pool,
            tc.tile_pool(name="ps", bufs=2, space="PSUM") as ps_pool,
        ):
            # Stage matmul lhsT through a DVE copy: matmult sem-wait encoding
            # is narrow, DMA completions fan out over many queue sems.
            e2t_st = const_pool.tile([PR, PR], BF16, tag="e2t_st")
            nc.sync.dma_start(e2t_st[:], e2t_dram[:])
            e2t = const_pool.tile([PR, PR], BF16, tag="e2t")
            nc.vector.tensor_copy(e2t[:], e2t_st[:])
            s0_st = const_pool.tile([PR, WCOL], BF16, tag="s0_st")
            nc.sync.dma_start(s0_st[:], s0_dram[:])

            # initial cohort states first: the gpsimd queue below must not
            # delay the memsets that gate the first matmul
            cur = []
            for k in range(NCOH):
                st = state_pool.tile([PR, CW], BF16, tag=f"w{k}")
                nc.gpsimd.memset(st[:], 1.0)
                if k == 0:
                    nc.vector.tensor_copy(st[:, 0:WCOL], s0_st[:])
                cur.append(st)

            # Spread G DMAs over the engine queues: sync and scalar HWDGE
            # queues share DMA engines E64-67 (~90 GB/s combined), the gpsimd
            # SWDGE queue stripes over E68-75 — give it the late big pieces
            # (its descriptor generation takes ~10-20 us on the Q7).
            gtiles = [[] for _ in range(NCOH)]
            for p in range(len(PIECES)):
                for k in range(NCOH):
                    off = sum(PIECES[:p])
                    nb = PIECES[p]
                    gt = g_pool.tile([PR, nb * CW], BF16, tag=f"g{k}_{p}")
                    if p >= len(PIECES) - 2:
                        eng = nc.gpsimd
                    else:
                        eng = nc.sync if k == 0 else nc.scalar
                    eng.dma_start(gt[:], g_dram[k][:, off * CW:(off + nb) * CW])
                    gtiles[k].append(gt)

            def gslice(k, i):
                for p in range(len(PIECES)):
                    if i < PIECES[p]:
                        return gtiles[k][p][:, i * CW:(i + 1) * CW]
                    i -= PIECES[p]
                raise AssertionError

            snaps = strip_pool.tile([PR, NSLOT * WCOL], BF16, tag="snaps")

            for i in range(TICKS):
                for k in range(NCOH):
                    ps = ps_pool.tile([PR, CW], F32, tag=f"s{k}")
                    nc.tensor.matmul(ps[:], e2t[:], cur[k][:],
                                     start=True, stop=True)
                    nw = state_pool.tile([PR, CW], BF16, tag=f"w{k}")
                    nc.vector.tensor_mul(nw[:], gslice(k, i), ps[:])
                    cur[k] = nw
                    if i == WARM - 1:
                        # start snapshots: chunks 1..11 (skip chunk 0) -> slots c-1
                        lo = 1 if k == 0 else 0
                        s0_slot = k * CPC + lo - 1
                        n_sl = CPC - lo
                        nc.scalar.copy(
                            snaps[:, s0_slot * WCOL:(s0_slot + n_sl) * WCOL],
                            nw[:, lo * WCOL:CPC * WCOL],
                        )
                        if k == NCOH - 1:
                            # ship start snaps now; end slots go at the end
                            nc.scalar.dma_start(
                                snaps_dram[:, 0:(C - 1) * WCOL],
                                snaps[:, 0:(C - 1) * WCOL],
                            )
                    if i == TICKS - 1:
                        # end snapshots: all chunks, slots 11..22
                        base = (C - 1) + k * CPC
                        nc.scalar.copy(
                            snaps[:, base * WCOL:(base + CPC) * WCOL], nw[:]
                        )

            nc.sync.dma_start(
                snaps_dram[:, (C - 1) * WCOL:], snaps[:, (C - 1) * WCOL:]
            )

    nc.compile()
    return nc


_NC_CACHE = {}


def _get_nc():
    if "nc" not in _NC_CACHE:
        _NC_CACHE["nc"] = _build_nc()
    return _NC_CACHE["nc"]


def _prep_inputs(logits, lens, transitions):
    """Host-side: exp/softmax prescale, park-row absorb rewrite, cohort packing."""
    logits = np.asarray(logits, np.float32)
    lens = np.asarray(lens, np.int64)
    T = np.asarray(transitions, np.float64)

    E = np.exp(T)
    Ebar = np.zeros((LBAR, LBAR), np.float64)
    Ebar[:L, :L] = E
    Ebar[PARK, :L] = E[STOP, :]
    Ebar[PARK, PARK] = 1.0

    e2t = np.zeros((PR, PR), np.float32)
    e2t[:LBAR, :LBAR] = Ebar.T
    e2t[LBAR:, LBAR:] = Ebar.T

    mx = logits.max(axis=2, keepdims=True)
    sumexp = np.exp(logits - mx).sum(axis=2)
    lse = mx[..., 0] + np.log(sumexp)                     # [B, S]
    sm = np.exp(logits - mx) / sumexp[..., None]          # [B, S, L]
    pbar = (Ebar[:L, :L] @ (np.ones(L) / L)).astype(np.float32)
    gamma = float(np.log(sm @ pbar).mean())

    active = np.arange(S)[None, :] < lens[:, None]        # [B, S]
    Gt = np.zeros((B, TS, LBAR), np.float32)
    Gt[:, :S, :L] = np.where(active[..., None], sm * np.float32(np.exp(-gamma)), 0.0)
    Gt[:, :S, PARK] = np.where(active, 0.0, 1.0)
    Gt[:, S, PARK] = 1.0

    corr = np.where(active, lse.astype(np.float64) + gamma, 0.0).sum(axis=1)

    state0 = Gt[:, 0, :] * Ebar[:, START].astype(np.float32)[None, :]  # [B, LBAR]

    # per-chunk app index at tick i (clamped to the pad absorb app TS-1... TS)
    app_idx = np.empty((C, TICKS), np.int64)
    for c in range(C):
        t0 = BOUNDS[c] - (0 if c == 0 else WARM)
        app_idx[c] = np.minimum(t0 + np.arange(TICKS), TS - 1)
        # chunk 11's final pad tick reuses the absorb app TS-1 (exact no-op)

    e2t_b = e2t.astype(ml_dtypes.bfloat16)
    in_maps = []
    for cc in range(NCORES):
        sl = slice(cc * BPC, (cc + 1) * BPC)
        # [128, TS, 46] -> [2, 46, TS, 64] -> [92, TS, 64]
        arr = np.transpose(
            Gt[sl].reshape(NG, WCOL, TS, LBAR), (0, 3, 2, 1)
        ).reshape(PR, TS, WCOL)
        s0 = np.ascontiguousarray(np.transpose(
            state0[sl].reshape(NG, WCOL, LBAR), (0, 2, 1)
        ).reshape(PR, WCOL)).astype(ml_dtypes.bfloat16)
        m = {"e2t": e2t_b, "s0": s0}
        for k in range(NCOH):
            # [92, TICKS, CPC, 64]: tick-major, chunk slices side by side
            chunks = app_idx[k * CPC:(k + 1) * CPC]       # [CPC, TICKS]
            blocks = arr[:, chunks.T]                     # [92, TICKS, CPC, 64]
            m[f"g{k}"] = np.ascontiguousarray(
                blocks.reshape(PR, TICKS * CW)
            ).astype(ml_dtypes.bfloat16)
        in_maps.append(m)
    return in_maps, corr, lens


def _postprocess(results, corr, lens):
    norm = np.empty(B, np.float64)
    for cc in range(NCORES):
        sn = np.asarray(results[cc]["snaps"]).astype(np.float64)
        sn = sn.reshape(PR, NSLOT, WCOL)
        for g in range(NG):
            rows = sn[g * LBAR:(g + 1) * LBAR]           # [46, NSLOT, 64]
            s = rows.sum(axis=0)                          # [NSLOT, 64]
            # slots: 0..10 = start snaps of chunks 1..11; 11..22 = end snaps
            logz = np.log(s[C - 1])                       # chunk 0 end
            for c in range(1, C - 1):
                logz += np.log(s[C - 1 + c]) - np.log(s[c - 1])
            park = rows[PARK, NSLOT - 1]                  # final state's park row
            logz += np.log(park) - np.log(s[C - 2])
            sl = slice(cc * BPC + g * WCOL, cc * BPC + (g + 1) * WCOL)
            norm[sl] = logz + corr[sl]
    return norm.astype(np.float32)


def kernel(logits, lens, transitions):
    nc = _get_nc()
    in_maps, corr, lens64 = _prep_inputs(logits, lens, transitions)
    res = run_bass_kernel_spmd(nc, in_maps, list(range(NCORES)))
    return _postprocess(res.results, corr, lens64)


# revision 16
# speedup vs baseline: 1.1894x; 1.1894x over previous
"""CRF forward (log partition) on 8 NeuronCores — chunked-parallel recurrence.

Math: the probability-space recurrence P_{t+1} = G_t o (E @ P_t) contracts
direction exponentially fast (products of positive matrices), so the 512
serial steps are split into C=12 time chunks run as independent streams,
each warm-started ~9 steps early from an all-ones state.  Host-side
stitching recovers log Z from per-chunk boundary row-sum ratios (the warmup
constant cancels); measured direction error after 8 steps is ~1e-5.

Range control without on-device renorm: active emission rows are prescaled
host-side by softmax times e^{-gamma}; the exact correction sum_t (LSE +
gamma) is added back on the host.  Absorbed steps (t >= len) park the
sequence's STOP projection in a dedicated 46th row per group whose
self-transition is exactly 1.0, so parked values are bit-stable in bf16.

Execution: 12 chunks form 2 lockstep cohorts of 6.  A cohort tick is ONE
PE matmul (lhsT = blockdiag(Ebar^T, Ebar^T) bf16, rhs = [92, 6*64] packed
states) and ONE DVE multiply (G-slice o PSUM -> next states), so the
PSUM-access cost and matmul fixed latency amortize over 6 chunks, and the
two cohorts keep PE and DVE pipelined against each other.
"""

import numpy as np
import ml_dtypes

import concourse.bacc as bacc
import concourse.bass as bass
import concourse.mybir as mybir
import concourse.tile as tile
from concourse.bass_utils import run_bass_kernel_spmd

L = 45
START = 43
STOP = 44
LBAR = 46                  # labels + park row
PARK = 45
B = 1024
S = 512
NCORES = 8
BPC = B // NCORES          # 128 sequences per core
NG = 2                     # groups per core
WCOL = BPC // NG           # 64 columns per group
PR = NG * LBAR             # 92 partition rows for packed state
TS = S + 1                 # apps 0..512 (app 0 folded host-side, 512 appended absorb)

C = 12                     # time chunks
NCOH = 2                   # lockstep cohorts
CPC = C // NCOH            # chunks per cohort
TICKS = 51                 # apps per chunk incl warmup
WARM = 9                   # warmup apps (chunks 1..C-1)
# windows: chunk 0 runs apps 1..51 exactly; chunks 1..10 cover 42 apps each,
# chunk 11 covers 41 apps + 1 pad absorb app (exact no-op on parked state).
W0 = TICKS
WC = 42
BOUNDS = [1, 1 + W0] + [1 + W0 + WC * c for c in range(1, C - 1)] + [TS]
assert BOUNDS[-2] + WC >= TS and len(BOUNDS) == C + 1

NSLOT = 2 * C - 1          # 11 start snaps + 12 end snaps
CW = CPC * WCOL            # cohort tile width (384)

F32 = mybir.dt.float32
BF16 = mybir.dt.bfloat16
FP8 = mybir.dt.float8e4

# DMA pieces per cohort G tensor, in ticks
PIECES = (3, 10, 38)


def _build_nc():
    nc = bacc.Bacc("TRN2", target_bir_lowering=False, debug=False, num_devices=NCORES)
    e2t_dram = nc.dram_tensor("e2t", [PR, PR], BF16, kind="ExternalInput")
    s0_dram = nc.dram_tensor("s0", [PR, WCOL], BF16, kind="ExternalInput")
    g_dram = [
        nc.dram_tensor(f"g{k}", [PR, TICKS * CW], FP8, kind="ExternalInput")
        for k in range(NCOH)
    ]
    snaps_dram = nc.dram_tensor("snaps", [PR, NSLOT * WCOL], BF16,
                                kind="ExternalOutput")

    with tile.TileContext(nc) as tc:
        with (
            tc.tile_pool(name="const", bufs=1) as const_pool,
            tc.tile_pool(name="gtiles", bufs=1) as g_pool,
            tc.tile_pool(name="strip", bufs=1) as strip_pool,
            tc.tile_pool(name="state", bufs=3) as state_pool,
            tc.tile_pool(name="ps", bufs=2, space="PSUM") as ps_pool,
        ):
            # Stage matmul lhsT through a DVE copy: matmult sem-wait encoding
            # is narrow, DMA completions fan out over many queue sems.
            e2t_st = const_pool.tile([PR, PR], BF16, tag="e2t_st")
            nc.sync.dma_start(e2t_st[:], e2t_dram[:])
            e2t = const_pool.tile([PR, PR], BF16, tag="e2t")
            nc.vector.tensor_copy(e2t[:], e2t_st[:])
            s0_st = const_pool.tile([PR, WCOL], BF16, tag="s0_st")
            nc.scalar.dma_start(s0_st[:], s0_dram[:])

            # initial cohort states first: the gpsimd queue below must not
            # delay the memsets that gate the first matmul
            cur = []
            for k in range(NCOH):
                st = state_pool.tile([PR, CW], BF16, tag=f"w{k}")
                nc.gpsimd.memset(st[:], 1.0)
                if k == 0:
                    nc.vector.tensor_copy(st[:, 0:WCOL], s0_st[:])
                cur.append(st)

            # Spread G DMAs over the engine queues: sync and scalar HWDGE
            # queues share DMA engines E64-67 (~90 GB/s combined), the gpsimd
            # SWDGE queue stripes over E68-75 — give it the late big pieces
            # (its descriptor generation takes ~10-20 us on the Q7).
            gtiles = [[] for _ in range(NCOH)]
            for p in range(len(PIECES)):
                for k in range(NCOH):
                    off = sum(PIECES[:p])
                    nb = PIECES[p]
                    gt = g_pool.tile([PR, nb * CW], FP8, tag=f"g{k}_{p}")
                    if p == len(PIECES) - 1:
                        eng = nc.gpsimd
                    else:
                        eng = nc.sync if k == 0 else nc.scalar
                    eng.dma_start(gt[:], g_dram[k][:, off * CW:(off + nb) * CW])
                    gtiles[k].append(gt)

            def gslice(k, i):
                for p in range(len(PIECES)):
                    if i < PIECES[p]:
                        return gtiles[k][p][:, i * CW:(i + 1) * CW]
                    i -= PIECES[p]
                raise AssertionError

            snaps = strip_pool.tile([PR, NSLOT * WCOL], BF16, tag="snaps")

            for i in range(TICKS):
                for k in range(NCOH):
                    ps = ps_pool.tile([PR, CW], F32, tag=f"s{k}")
                    nc.tensor.matmul(ps[:], e2t[:], cur[k][:],
                                     start=True, stop=True)
                    nw = state_pool.tile([PR, CW], BF16, tag=f"w{k}")
                    nc.vector.tensor_mul(nw[:], gslice(k, i), ps[:])
                    cur[k] = nw
                    if i == WARM - 1:
                        # start snapshots: chunks 1..11 (skip chunk 0) -> slots c-1
                        lo = 1 if k == 0 else 0
                        s0_slot = k * CPC + lo - 1
                        n_sl = CPC - lo
                        nc.scalar.copy(
                            snaps[:, s0_slot * WCOL:(s0_slot + n_sl) * WCOL],
                            nw[:, lo * WCOL:CPC * WCOL],
                        )
                        if k == NCOH - 1:
                            # ship start snaps now; end slots go at the end
                            nc.scalar.dma_start(
                                snaps_dram[:, 0:(C - 1) * WCOL],
                                snaps[:, 0:(C - 1) * WCOL],
                            )
                    if i == TICKS - 1:
                        # end snapshots: all chunks, slots 11..22
                        base = (C - 1) + k * CPC
                        nc.scalar.copy(
                            snaps[:, base * WCOL:(base + CPC) * WCOL], nw[:]
                        )

            nc.sync.dma_start(
                snaps_dram[:, (C - 1) * WCOL:], snaps[:, (C - 1) * WCOL:]
            )

    nc.compile()
    return nc


_NC_CACHE = {}


def _get_nc():
    if "nc" not in _NC_CACHE:
        _NC_CACHE["nc"] = _build_nc()
    return _NC_CACHE["nc"]


def _prep_inputs(logits, lens, transitions):
    """Host-side: exp/softmax prescale, park-row absorb rewrite, cohort packing."""
    logits = np.asarray(logits, np.float32)
    lens = np.asarray(lens, np.int64)
    T = np.asarray(transitions, np.float64)

    E = np.exp(T)
    Ebar = np.zeros((LBAR, LBAR), np.float64)
    Ebar[:L, :L] = E
    Ebar[PARK, :L] = E[STOP, :]
    Ebar[PARK, PARK] = 1.0

    e2t = np.zeros((PR, PR), np.float32)
    e2t[:LBAR, :LBAR] = Ebar.T
    e2t[LBAR:, LBAR:] = Ebar.T

    mx = logits.max(axis=2, keepdims=True)
    sumexp = np.exp(logits - mx).sum(axis=2)
    lse = mx[..., 0] + np.log(sumexp)                     # [B, S]
    sm = np.exp(logits - mx) / sumexp[..., None]          # [B, S, L]
    pbar = (Ebar[:L, :L] @ (np.ones(L) / L)).astype(np.float32)
    gamma = float(np.log(sm @ pbar).mean())

    active = np.arange(S)[None, :] < lens[:, None]        # [B, S]
    Gt = np.zeros((B, TS, LBAR), np.float32)
    Gt[:, :S, :L] = np.where(active[..., None], sm * np.float32(np.exp(-gamma)), 0.0)
    Gt[:, :S, PARK] = np.where(active, 0.0, 1.0)
    Gt[:, S, PARK] = 1.0

    corr = np.where(active, lse.astype(np.float64) + gamma, 0.0).sum(axis=1)

    state0 = Gt[:, 0, :] * Ebar[:, START].astype(np.float32)[None, :]  # [B, LBAR]

    # per-chunk app index at tick i (clamped to the pad absorb app TS-1... TS)
    app_idx = np.empty((C, TICKS), np.int64)
    for c in range(C):
        t0 = BOUNDS[c] - (0 if c == 0 else WARM)
        app_idx[c] = np.minimum(t0 + np.arange(TICKS), TS - 1)
        # chunk 11's final pad tick reuses the absorb app TS-1 (exact no-op)

    e2t_b = e2t.astype(ml_dtypes.bfloat16)
    in_maps = []
    for cc in range(NCORES):
        sl = slice(cc * BPC, (cc + 1) * BPC)
        # [128, TS, 46] -> [2, 46, TS, 64] -> [92, TS, 64]
        arr = np.transpose(
            Gt[sl].reshape(NG, WCOL, TS, LBAR), (0, 3, 2, 1)
        ).reshape(PR, TS, WCOL)
        s0 = np.ascontiguousarray(np.transpose(
            state0[sl].reshape(NG, WCOL, LBAR), (0, 2, 1)
        ).reshape(PR, WCOL)).astype(ml_dtypes.bfloat16)
        m = {"e2t": e2t_b, "s0": s0}
        for k in range(NCOH):
            # [92, TICKS, CPC, 64]: tick-major, chunk slices side by side
            chunks = app_idx[k * CPC:(k + 1) * CPC]       # [CPC, TICKS]
            blocks = arr[:, chunks.T]                     # [92, TICKS, CPC, 64]
            m[f"g{k}"] = np.ascontiguousarray(
                blocks.reshape(PR, TICKS * CW)
            ).astype(ml_dtypes.float8_e4m3fn)
        in_maps.append(m)
    return in_maps, corr, lens


def _postprocess(results, corr, lens):
    norm = np.empty(B, np.float64)
    for cc in range(NCORES):
        sn = np.asarray(results[cc]["snaps"]).astype(np.float64)
        sn = sn.reshape(PR, NSLOT, WCOL)
        for g in range(NG):
            rows = sn[g * LBAR:(g + 1) * LBAR]           # [46, NSLOT, 64]
            s = rows.sum(axis=0)                          # [NSLOT, 64]
            # slots: 0..10 = start snaps of chunks 1..11; 11..22 = end snaps
            logz = np.log(s[C - 1])                       # chunk 0 end
            for c in range(1, C - 1):
                logz += np.log(s[C - 1 + c]) - np.log(s[c - 1])
            park = rows[PARK, NSLOT - 1]                  # final state's park row
            logz += np.log(park) - np.log(s[C - 2])
            sl = slice(cc * BPC + g * WCOL, cc * BPC + (g + 1) * WCOL)
            norm[sl] = logz + corr[sl]
    return norm.astype(np.float32)


def kernel(logits, lens, transitions):
    nc = _get_nc()
    in_maps, corr, lens64 = _prep_inputs(logits, lens, transitions)
    res = run_bass_kernel_spmd(nc, in_maps, list(range(NCORES)))
    return _postprocess(res.results, corr, lens64)
